# revision 1
# baseline (speedup 1.0000x reference)
# Trainium2 Bass kernel for nn_ExpandFrame: gaussian-upsampling attention
#   e = cumsum(duration, -1); c = e - 0.5*round(duration)
#   logits[b,n,t] = temp * (t - c[b,n])^2 ;  temp = -1/(5*sqrt(duration[0,0]))
#   w = softmax(logits, axis=n) ;  out[b,d,t] = sum_n w[b,n,t] * hidden[b,n,d]
#
# Strategy: data-parallel over batch B=16 across 8 cores (2 batches/core).
# The softmax weights form a narrow band (|t - c_n| <~ 30), so both the
# softmax and the contraction run over host-computed static n-windows
# (aligned 128-chunks), shared by all batches so one SPMD program serves
# all cores. Softmax is computed in [t_partition, n_free] layout (free-axis
# reductions), transposed on the PE to [n,t] for the banded matmul
# (float32r = full-rate fp32), accumulated in PSUM, copied out and DMA'd.
import numpy as np

B, N, D, T = 16, 1024, 1024, 4096
NCORES = 8
BPC = B // NCORES        # batches per core
P = 128                  # partitions
TT = 512                 # matmul t-tile (PSUM bank = 512 fp32)
NTT = T // TT            # 8
TC = 128                 # softmax t-chunk (one partition block)
NTC = T // TC            # 32
KN = N // P              # 8 n-chunks

MATMUL_MODE = "f32r"     # "f32r" | "f32"


def _host_prep(duration):
    """Centers, temp, and static band windows (shared across all batches)."""
    dur = np.asarray(duration, dtype=np.float32)
    e = np.cumsum(dur, axis=-1, dtype=np.float32)
    c = (e - np.float32(0.5) * np.round(dur)).astype(np.float32)   # [B, N]
    d00 = float(dur[0, 0])
    temp = -1.0 / (5.0 * np.sqrt(d00))
    s = float(np.sqrt(-temp))
    margin = int(np.ceil(np.sqrt(60.0 / -temp))) + 2

    # per-(b, t-chunk) n-window, then uniform across batches
    lo = np.empty((B, NTC), dtype=np.int64)
    hi = np.empty((B, NTC), dtype=np.int64)
    for b in range(B):
        t0s = np.arange(NTC) * TC
        lo[b] = np.searchsorted(c[b], t0s - margin, side="left")
        hi[b] = np.searchsorted(c[b], t0s + (TC - 1) + margin, side="right")
    ulo = np.minimum(lo.min(axis=0), N - 1)
    uhi = np.maximum(hi.max(axis=0), ulo + 1)
    klo_tc = ulo // P                       # aligned chunk ranges per t-chunk
    khi_tc = (uhi + P - 1) // P
    # matmul windows per 512-t tile = union over its 4 chunks
    klo_tt = klo_tc.reshape(NTT, 4).min(axis=1)
    khi_tt = khi_tc.reshape(NTT, 4).max(axis=1)

    # which t-chunks need max-subtraction for stability (tail shortfall)
    need_min = np.zeros(NTC, dtype=bool)
    tgrid = np.arange(T, dtype=np.float32)
    for b in range(B):
        idx = np.searchsorted(c[b], tgrid)
        dl = np.abs(tgrid - c[b][np.clip(idx - 1, 0, N - 1)])
        dr = np.abs(c[b][np.clip(idx, 0, N - 1)] - tgrid)
        dmin = np.minimum(dl, dr)
        posmin = (-temp) * dmin * dmin
        need_min |= (posmin.reshape(NTC, TC).max(axis=1) > 25.0)

    tneg = (-s * (np.arange(NTC)[None, :] * TC + np.arange(P)[:, None])
            ).astype(np.float32)            # [P, NTC]
    return c, s, klo_tc, khi_tc, klo_tt, khi_tt, need_min, tneg


def _build(nc, klo_tc, khi_tc, klo_tt, khi_tt, need_min, s):
    import concourse.tile as tile
    import concourse.mybir as mybir
    from concourse import masks

    f32 = mybir.dt.float32
    AF = mybir.ActivationFunctionType
    ALU = mybir.AluOpType
    mm_dt = {"f32r": mybir.dt.float32r, "f32": f32,
             "bf16": mybir.dt.bfloat16}[MATMUL_MODE]

    hid = nc.dram_tensor("hidden", [BPC, N, D], f32, kind="ExternalInput").ap()
    cbd = nc.dram_tensor("cb", [BPC, N], f32, kind="ExternalInput").ap()
    outd = nc.dram_tensor("out", [BPC, D, T], f32, kind="ExternalOutput").ap()

    with tile.TileContext(nc) as tc:
        import contextlib
        with contextlib.ExitStack() as ctx:
            constp = ctx.enter_context(tc.tile_pool(name="const", bufs=1))
            hidp = ctx.enter_context(tc.tile_pool(name="hid", bufs=2))
            cbp = ctx.enter_context(tc.tile_pool(name="cbp", bufs=2))
            cbrp = ctx.enter_context(tc.tile_pool(name="cbr", bufs=2))
            softp = ctx.enter_context(tc.tile_pool(name="soft", bufs=12))
            wp = ctx.enter_context(tc.tile_pool(name="wp", bufs=12))
            statp = ctx.enter_context(tc.tile_pool(name="stat", bufs=32))
            wTp = ctx.enter_context(tc.tile_pool(name="wT", bufs=12))
            osbp = ctx.enter_context(tc.tile_pool(name="osb", bufs=10))
            ptp = ctx.enter_context(tc.tile_pool(name="pt", bufs=4, space="PSUM"))
            pop = ctx.enter_context(tc.tile_pool(name="po", bufs=4, space="PSUM"))

            tr_dt = mybir.dt.bfloat16  # w/transpose path dtype
            ident = constp.tile([P, P], tr_dt)
            masks.make_identity(nc, ident[:])
            # tneg[p, tc] = -s * (tc*128 + p), built on-chip via iota
            tneg_i = constp.tile([P, NTC], mybir.dt.int32)
            nc.gpsimd.iota(tneg_i[:], pattern=[[P, NTC]], base=0,
                           channel_multiplier=1)
            tneg_sb = constp.tile([P, NTC], f32)
            nc.scalar.mul(tneg_sb[:], tneg_i[:], -s)
            # warm the ACT spline tables before the hidden-DMA flood so the
            # table-load DMA isn't queued behind 4MB of input traffic
            warm = constp.tile([P, 1], f32)
            nc.scalar.activation(warm[:], tneg_sb[:, 0:1], AF.Square,
                                 bias=0.0, scale=1.0)
            nc.scalar.activation(warm[:], warm[:], AF.Exp,
                                 bias=0.0, scale=-1.0)

            for b in range(BPC):
                cb_row = cbrp.tile([1, N], f32, tag="cbr")
                nc.sync.dma_start(cb_row[:], cbd[b][None, :])
                cb_sb = cbp.tile([P, N], f32, tag="cb")
                nc.gpsimd.partition_broadcast(cb_sb[:], cb_row[:], channels=P)
                if MATMUL_MODE == "bf16":
                    hid_f32 = hidp.tile([P, KN, D], f32, tag="hidf")
                    hid_sb = hidp.tile([P, KN, D], mm_dt, tag="hid")
                    for k in range(KN):
                        nc.sync.dma_start(hid_f32[:, k, :],
                                          hid[b, k * P:(k + 1) * P, :])
                        nc.vector.tensor_copy(hid_sb[:, k, :], hid_f32[:, k, :])
                else:
                    hid_sb = hidp.tile([P, KN, D], mm_dt, tag="hid")
                    for k in range(KN):
                        nc.sync.dma_start(
                            hid_sb[:, k, :],
                            hid[b, k * P:(k + 1) * P, :].bitcast(mm_dt))

                for pr in range(NTT // 2):
                    # --- softmax + transpose for both t-tiles of the pair ---
                    pair_wT = []
                    pair_win = []
                    for tt in (2 * pr, 2 * pr + 1):
                        klo, khi = int(klo_tt[tt]), int(khi_tt[tt])
                        kw = khi - klo
                        nwin = kw * P
                        wtiles = []
                        for j in range(4):
                            tcid = tt * 4 + j
                            pos = softp.tile([P, nwin], f32, tag="pos")
                            nc.scalar.activation(
                                pos[:], cb_sb[:, klo * P: klo * P + nwin],
                                AF.Square, bias=tng_col(tneg_sb, tcid), scale=s)
                            p_t = softp.tile([P, nwin], tr_dt, tag="p")
                            s_col = statp.tile([P, 1], f32, tag="S")
                            if need_min[tcid]:
                                m_col = statp.tile([P, 1], f32, tag="m")
                                nc.vector.tensor_reduce(
                                    m_col[:], pos[:], axis=mybir.AxisListType.X,
                                    op=ALU.min)
                                nc.scalar.activation(
                                    p_t[:], pos[:], AF.Exp, bias=m_col[:],
                                    scale=-1.0, accum_out=s_col[:])
                            else:
                                nc.scalar.activation(
                                    p_t[:], pos[:], AF.Exp, bias=0.0,
                                    scale=-1.0, accum_out=s_col[:])
                            r_col = statp.tile([P, 1], f32, tag="r")
                            nc.vector.reciprocal(r_col[:], s_col[:])
                            # diag(r): transpose-with-scale via PE matmul
                            dg = wp.tile([P, P], tr_dt, tag="dg")
                            nc.vector.tensor_scalar_mul(dg[:], ident[:], r_col[:])
                            wtiles.append((p_t, dg))

                        wT = []
                        for ki in range(kw):
                            pt = ptp.tile([P, TT], f32, tag="pt")
                            for j in range(4):
                                p_t, dg = wtiles[j]
                                nc.tensor.matmul(
                                    pt[:, j * P:(j + 1) * P],
                                    p_t[:, ki * P:(ki + 1) * P],
                                    dg[:], start=True, stop=True)
                            wk = wTp.tile([P, TT], mm_dt, tag="wT")
                            nc.vector.tensor_copy(wk[:], pt[:])
                            wT.append(wk)
                        pair_wT.append(wT)
                        pair_win.append((klo, khi))

                    # --- banded matmuls, paired per d-chunk; one DMA per pair ---
                    for dci in range(D // P):
                        osb = osbp.tile([P, 2 * TT], f32, tag="osb")
                        for ti in range(2):
                            klo, khi = pair_win[ti]
                            kw = khi - klo
                            po = pop.tile([P, TT], f32, tag="po")
                            for ki, k in enumerate(range(klo, khi)):
                                nc.tensor.matmul(
                                    po[:],
                                    hid_sb[:, k, dci * P:(dci + 1) * P],
                                    pair_wT[ti][ki][:],
                                    start=(ki == 0), stop=(ki == kw - 1))
                            dst = osb[:, ti * TT:(ti + 1) * TT]
                            if (dci * 2 + ti) % 16 in (0, 3, 6, 9, 12):
                                nc.scalar.copy(dst, po[:])
                            else:
                                nc.vector.tensor_copy(dst, po[:])
                        nc.sync.dma_start(
                            outd[b, dci * P:(dci + 1) * P,
                                 pr * 2 * TT:(pr + 1) * 2 * TT],
                            osb[:])
    return nc


def tng_col(tneg_sb, tcid):
    return tneg_sb[:, tcid:tcid + 1]


def _run(inputs, trace=False):
    import concourse.bacc as bacc
    from concourse.bass_utils import run_bass_kernel_spmd

    hidden = np.ascontiguousarray(np.asarray(inputs["hidden"], dtype=np.float32))
    duration = np.asarray(inputs["duration"], dtype=np.float32)

    c, s, klo_tc, khi_tc, klo_tt, khi_tt, need_min, tneg = _host_prep(duration)

    nc = bacc.Bacc("TRN2", target_bir_lowering=False, debug=False,
                   enable_asserts=False, num_devices=NCORES)
    _build(nc, klo_tc, khi_tc, klo_tt, khi_tt, need_min, s)
    nc.compile()

    in_maps = []
    for i in range(NCORES):
        in_maps.append({
            "hidden": hidden[i * BPC:(i + 1) * BPC],
            "cb": np.ascontiguousarray(c[i * BPC:(i + 1) * BPC]),
        })
    res = run_bass_kernel_spmd(nc, in_maps, core_ids=list(range(NCORES)),
                               trace=trace)
    out = np.concatenate([res.results[i]["out"] for i in range(NCORES)], axis=0)
    return out, res


def kernel(**inputs) -> np.ndarray:
    out, _ = _run(inputs, trace=False)
    return out

